# revision 24
# baseline (speedup 1.0000x reference)
"""Trainium2 Bass kernel for nn_AttentionBase (8-head attention w/ T5-style
relative-position bias + output projection), sharded head-parallel over 8
NeuronCores.

v4: the n^2 softmax-exp elementwise work is split between the ACT engine
(exact spline exp, A-steps) and the DVE (custom fused fast-exp2 op,
D-steps, every 3rd step) so both engines stream PSUM score tiles in
parallel instead of ACT being the lone ~288us bottleneck of v1.

Host side (per core c, head h=c): q is pre-scaled by QS = 128*log2(e)*SCALE
so the QK PSUM output is in "bf16 bits" units; both q,k get a 65th
contraction row (qT=1, kT=16256) so st arrives with the +16256 bf16
exponent-bias offset pre-added (16256 = 127*128 is exact in bf16).
v is laid out as [2, 128, Kt*128] bf16 with a ones-column at 64
(softmax-denominator trick). Two Toeplitz tables:
  expP  [128, 2n-128] f16 : kappa * exp(SCALE*bias[key-query])   (A-steps)
  biasT [128, 2n-128] f16 : QS*bias[key-query] + delta           (D-steps)

Device program per core (single head, both batches), per step s=(qb, kt):
  S^T[128, w] = kT_tile^T.T @ qT             (PE, bf16 in, fp32 PSUM,
                                              3-deep st ring)
  A-step (2 of 3): araw = exp(S*SCALE/QS - 16256*SCALE/QS)  (ACT spline)
                   at = araw * expP_slice    (DVE tensor_tensor, 2x 16-bit)
  D-step (1 of 3): at.bits = EXP2BITS(S^T + biasT_slice)
                   (custom DVE op: u+=bias; round-to-128-multiple via +-2^30;
                    quadratic mantissa fit; int16 convert == exact bf16 bits
                    of kappa*exp(SCALE*(s+bias)); one 1x DVE pass, no ACT)
  outT[65, w] += V'[kt]^T.T @ at             (PE, lagged LAG steps)
Projections (o65 chunk @ W_h -> PSUM, two per shared 2-bank st-ring slot
every 8 steps) drain via ACT scalar.mul(1/den) several steps after their
matmul — and only once their 1/den reciprocal has had time to clear the DVE
queue — so the in-order ACT queue never stalls on an in-flight producer;
the kappa factor cancels in the softmax normalization.
Host: out = sum_c partial_c + b_out (float64 accumulate).
"""

import math
import sys

sys.path.insert(0, "/opt/trn_rl_repo")

import numpy as np
import ml_dtypes

import concourse.bass as bass
import concourse.bacc as bacc_mod
import concourse.mybir as mybir
import concourse.tile as tile

NUM_HEADS = 8
HEAD_DIM = 64
MID = 512
OUT_F = 512
NUM_BUCKETS = 32
MAX_DISTANCE = 128
SCALE = HEAD_DIM ** -0.5
N_CORES = 8

F32 = mybir.dt.float32
BF16 = mybir.dt.bfloat16
F16 = mybir.dt.float16
I16 = mybir.dt.int16
AF = mybir.ActivationFunctionType

# fast-exp2 constants (round-to-nearest-even int16 convert, HW-verified)
QS = 128 * math.log2(math.e) * SCALE        # 23.083120654223414 bits/unit
COFF = 16256.0                              # 127*128, exact in bf16
EXP2_DELTA = 24.04
EXP2_B = 0.0027580
EXP2_D = 53.08
EXP2_KAPPA = 1.611958
M30 = float(2.0 ** 30)
DK = 3                                       # every DK-th step is a D-step
KD = 65                                      # contraction: 64 dims + COFF row


def _register_exp2_op():
    """Idempotently register the custom DVE op (documented extension path:
    dve_ops.OPS + sub-opcode row; the per-NEFF uop table is generated at
    compile time, no firmware change)."""
    import concourse.dve_ops as dve_ops
    from concourse.dve_spec import Spec, Src0, Src1, C0, C1, C2

    for op in dve_ops.OPS:
        if op.name == "EXP2BITS_ANT":
            return op

    def ref(in0, in1, c0, c1, c2):
        c0v = np.float32(c0 if np.isscalar(c0) else np.asarray(c0).ravel()[0])
        c1v = np.float32(c1 if np.isscalar(c1) else np.asarray(c1).ravel()[0])
        u = (in0.astype(np.float32) + in1.astype(np.float32)).astype(np.float32)
        t = (u + c0v).astype(np.float32)
        i = (t - c0v).astype(np.float32)
        f = (u - i).astype(np.float32)
        p = ((np.float32(c2) * f) * f + c1v).astype(np.float32)
        return (u + p).astype(np.float32)

    u = Src0 + Src1
    t = u + C0
    i = t - C0
    f = u - i
    body = u + ((C2 * f) * f + C1)
    op = dve_ops.DveOp(
        "EXP2BITS_ANT",
        dve_ops.Spec(body=body, reference=ref),
        subdim=False,
        uops_sha={"v3": "3f0b102a66e52a77"},
    )
    dve_ops.OPS.append(op)
    dve_ops.CUSTOM_DVE_SPECS[op.name] = op.spec
    dve_ops._SUB_OPCODE_FOR_NAME[op.name] = (
        dve_ops._CUSTOM_DVE_ROW_BASE + len(dve_ops.OPS) - 1
    )
    assert dve_ops._SUB_OPCODE_FOR_NAME[op.name] < 0x20
    return op


EXP2_OP = _register_exp2_op()


def _bucket_np(rel):
    """Exact numpy port of reference._relative_position_bucket with
    num_buckets=64, max_distance=128 (as the module calls it)."""
    num_buckets = (2 * NUM_BUCKETS) // 2  # 32
    ret = (rel >= 0).astype(np.int32) * num_buckets
    n = np.abs(rel)
    max_exact = max(1, num_buckets // 2)  # 16
    denom = (
        math.log(MAX_DISTANCE / max_exact) if MAX_DISTANCE > max_exact else 1.0
    )
    n_float = np.maximum(n.astype(np.float32), 1.0)
    val_if_large = (
        max_exact + np.log(n_float / max_exact) / denom * (num_buckets - max_exact)
    ).astype(np.int32)
    val_if_large = np.minimum(val_if_large, num_buckets - 1)
    return ret + np.where(n < max_exact, n, val_if_large)


def _toeplitz_tables(table_col, n):
    """expP (f16) and biasT (f16) [128, 2n-128] Toeplitz tables. Slice
    [:, c0:c0+w] with c0 = (n-128) - 128*kt + w*qb gives the [key, query]
    tile."""
    x = np.arange(2 * n, dtype=np.int64)
    d = (n - 1) - x
    bias = table_col[_bucket_np(d)].astype(np.float64)
    expP_seq = (EXP2_KAPPA * np.exp(SCALE * bias)).astype(np.float16)
    biasT_seq = (QS * bias + EXP2_DELTA).astype(np.float16)
    Wp = 2 * n - 128
    idx = (127 - np.arange(128, dtype=np.int64))[:, None] + np.arange(
        Wp, dtype=np.int64
    )[None, :]
    return expP_seq[idx], biasT_seq[idx]


def build_nc(n=4096, w=1024):
    assert n % 128 == 0 and n % w == 0
    Kt = n // 128           # key tiles
    n_qb = n // w           # query blocks per batch
    nsteps = n_qb * Kt      # attention steps per batch
    qb_t = w // 128         # query tiles per query block
    Wp = 2 * n - 128
    LAG = 4                 # AV trails QK by this many steps on the PE queue
    VW = HEAD_DIM + 1       # V' valid width (ones column at 64)
    VP = 128                # V' padded width: 128 weights enables FWL

    nc = bacc_mod.Bacc()
    qT_h = nc.declare_dram_parameter("qT_h", [2, KD, n], BF16, isOutput=False)
    kT_h = nc.declare_dram_parameter("kT_h", [2, KD, n], BF16, isOutput=False)
    vp_h = nc.declare_dram_parameter("vp_h", [2, 128, Kt * VP], BF16, isOutput=False)
    expP_h = nc.declare_dram_parameter("expP_h", [128, Wp], F16, isOutput=False)
    biasT_h = nc.declare_dram_parameter("biasT_h", [128, Wp], F16, isOutput=False)
    W_h = nc.declare_dram_parameter("W_h", [HEAD_DIM, OUT_F], BF16, isOutput=False)
    out_p = nc.declare_dram_parameter(
        "out_partial", [2, n, OUT_F], BF16, isOutput=True
    )
    den_scr = nc.dram_tensor("den_scr", (2, n), F32)

    act_scale = float(SCALE / QS)
    act_bias = float(-COFF * SCALE / QS)

    def is_d_step(b, s):
        return s % DK == DK - 1

    with tile.TileContext(nc) as tc:
        with (
            tc.tile_pool(name="const", bufs=1) as constp,
            tc.tile_pool(name="qkT", bufs=2) as qkTp,
            tc.tile_pool(name="vpp", bufs=2) as vpp,
            tc.tile_pool(name="o65p", bufs=2) as o65p,
            tc.tile_pool(name="stage", bufs=6) as stage,
            tc.tile_pool(name="atp", bufs=LAG + 4) as atp,
            tc.tile_pool(name="osbp", bufs=6) as osbp,
            tc.tile_pool(name="recp", bufs=3) as recp,
            tc.tile_pool(name="pst", bufs=3, space="PSUM") as pst,
            tc.tile_pool(name="pav", bufs=1, space="PSUM") as pav,
        ):
            # PE warm-up burst: a few dependency-free matmuls on a memset
            # tile keep the PE fed while the first input chunks stream in
            warm = constp.tile([HEAD_DIM, 512], BF16, tag="warm")
            nc.gpsimd.memset(warm, 0.0)
            # per-partition constant for the A-step ACT bias (-COFF*SCALE/QS)
            actb = constp.tile([128, 1], F32, tag="actb")
            nc.gpsimd.memset(actb, act_bias)
            wpsum = pst.tile([128, 512], F32, tag="st", name="wpsum")
            for _ in range(4):
                nc.tensor.matmul(
                    wpsum, warm[:, 0:128], warm[:, :], start=True, stop=True
                )

            # Toeplitz tables stream on the sync queue in first-use order
            # (the first steps read cols ~[n-128-256, n-128+w)); chunked so
            # the first A/D steps wait only on their own chunk.
            expP = constp.tile([128, Wp], F16, tag="expP")
            biasT = constp.tile([128, Wp], F16, tag="biasT")
            # first-use priority: qb0 reads cols [n-128+w) downward from
            # c0 = n-128-128*kt, so the top-of-range slices gate step 0
            hi0 = n - 128 + w
            mid = (hi0 + Wp) // 2
            for c0, c1 in (
                (n - 512, hi0),
                (n // 2, n - 512),
                (0, n // 2),
                (hi0, mid),
                (mid, Wp),
            ):
                cs = slice(c0, c1)
                nc.sync.dma_start(expP[:, cs], expP_h[:, cs])
                nc.sync.dma_start(biasT[:, cs], biasT_h[:, cs])
            Wt = constp.tile([HEAD_DIM, OUT_F], BF16, tag="W")
            nc.sync.dma_start(Wt, W_h[:, :])

            proj_jobs = []   # pending projections (args tuples)
            drain_jobs = []  # (enqueue_step, drain closure)
            den_jobs = []    # deferred final-block denominator closure
            cur_s = [10 ** 9]  # current step, for drain readiness

            def make_den_pe(b, qb, o65, vp):
                # transpose the [1, w] denominator row to [128, qb_t] with
                # qb_t rank-1 PE matmuls — used only for the very last query
                # block, where the DRAM bounce's serial latency would sit
                # entirely in the kernel tail
                def issue():
                    denP = pst.tile([128, qb_t], F32, tag="st", name="denP")
                    for j in range(qb_t):
                        nc.tensor.matmul(
                            denP[:, j : j + 1],
                            o65[
                                HEAD_DIM : HEAD_DIM + 1,
                                w * qb + 128 * j : w * qb + 128 * (j + 1),
                            ],
                            vp[
                                HEAD_DIM : HEAD_DIM + 1,
                                HEAD_DIM : HEAD_DIM + 1,
                            ],
                            start=True,
                            stop=True,
                        )
                    denT = recp.tile([128, qb_t], F32, tag="denT")
                    nc.vector.tensor_copy(denT, denP)
                    rec = recp.tile([128, qb_t], F32, tag="rec")
                    nc.vector.reciprocal(rec, denT)
                    # tail: issue all MMs first, then drains (alternating
                    # engines) so no drain waits on an in-flight MM
                    local = []
                    for rj in range(qb_t):
                        local.append(
                            make_pj(
                                b, qb_t * qb + rj, o65, rec, rj,
                                on_dve=rj % 2 == 1,
                            )
                        )
                    for j, mm in enumerate(local):
                        mm()
                        if j >= 1:
                            drain_jobs.pop(0)[1]()
                    while drain_jobs:
                        drain_jobs.pop(0)[1]()

                return issue

            def make_pj(b, qt, o65, rec, rj, on_dve=False, pair=None):
                """Projection for query tile qt. `pair`: (pj_tile, half) to
                share one 2-bank PSUM slot between two projections."""

                def issue_mm():
                    if pair is None:
                        pj = pst.tile([128, OUT_F], F32, tag="st")
                        lo = 0
                    else:
                        pjt, half = pair
                        pj = pjt
                        lo = OUT_F * half
                    nc.tensor.matmul(
                        pj[:, lo : lo + OUT_F],
                        o65[0:HEAD_DIM, 128 * qt : 128 * (qt + 1)],
                        Wt,
                        start=True,
                        stop=True,
                    )

                    def issue_drain():
                        osb = osbp.tile([128, OUT_F], BF16, tag="osb")
                        src = pj[:, lo : lo + OUT_F]
                        if on_dve:
                            nc.vector.tensor_scalar_mul(
                                osb, src, rec[:, rj : rj + 1]
                            )
                        else:
                            # steady state: ACT has slack, DVE is busier
                            nc.scalar.mul(osb, src, rec[:, rj : rj + 1])
                        nc.sync.dma_start(
                            out_p[b, 128 * qt : 128 * (qt + 1), :], osb
                        )

                    drain_jobs.append((cur_s[0], issue_drain))

                return issue_mm

            for b in range(2):
                # loads ordered by first use: the first key tiles and the
                # first query block gate step 0; the rest streams behind
                qT = qkTp.tile([KD, n], BF16, tag="qT")
                kT = qkTp.tile([KD, n], BF16, tag="kT")
                nc.gpsimd.dma_start(kT[:, 0:512], kT_h[b][:, 0:512])
                nc.gpsimd.dma_start(qT[:, 0:w], qT_h[b][:, 0:w])
                nc.gpsimd.dma_start(kT[:, 512:2048], kT_h[b][:, 512:2048])
                nc.gpsimd.dma_start(kT[:, 2048:n], kT_h[b][:, 2048:n])
                vp = vpp.tile([128, Kt * VP], BF16, tag="vp")
                nc.gpsimd.dma_start(
                    vp[:, 0 : Kt * VP // 4], vp_h[b][:, 0 : Kt * VP // 4]
                )
                if n > w:
                    nc.gpsimd.dma_start(qT[:, w:n], qT_h[b][:, w:n])
                for ch in range(1, 4):
                    cs = slice(ch * Kt * VP // 4, (ch + 1) * Kt * VP // 4)
                    nc.gpsimd.dma_start(vp[:, cs], vp_h[b][:, cs])

                o65 = o65p.tile([VW, n], BF16, tag="o65")
                ats = {}
                avs = {}

                def issue_qk(s, b=b, qT=qT, kT=kT, ats=ats):
                    qb, kt = divmod(s, Kt)
                    st = pst.tile([128, w], F32, tag="st")
                    for h in range(w // 512):
                        nc.tensor.matmul(
                            st[:, 512 * h : 512 * (h + 1)],
                            kT[:, 128 * kt : 128 * (kt + 1)],
                            qT[:, w * qb + 512 * h : w * qb + 512 * (h + 1)],
                            start=True,
                            stop=True,
                        )
                    at = atp.tile([128, w], BF16, tag="at")
                    c0 = (n - 128) - 128 * kt + w * qb
                    if is_d_step(b, s):
                        nc.vector._custom_dve(
                            EXP2_OP,
                            out=at[:, :].bitcast(I16),
                            in0=st[:, :],
                            in1=biasT[:, c0 : c0 + w],
                            s0=M30,
                            s1=EXP2_D,
                            imm2=EXP2_B,
                        )
                    else:
                        araw = stage.tile([128, w], BF16, tag="araw")
                        nc.scalar.activation(
                            araw, st, AF.Exp, bias=actb[:, 0:1], scale=act_scale
                        )
                        nc.vector.tensor_mul(at, araw, expP[:, c0 : c0 + w])
                    ats[s] = at

                def issue_av(s, b=b, vp=vp, o65=o65, ats=ats, avs=avs):
                    qb, kt = divmod(s, Kt)
                    if kt == 0:
                        avs[qb] = pav.tile([128, w], F32, tag="av", name="av")
                    av = avs[qb]
                    at = ats.pop(s)
                    for h in range(w // 512):
                        nc.tensor.matmul(
                            av[:, 512 * h : 512 * (h + 1)],
                            vp[:, VP * kt : VP * (kt + 1)],
                            at[:, 512 * h : 512 * (h + 1)],
                            start=(kt == 0),
                            stop=(kt == Kt - 1),
                        )
                    if kt == Kt - 1:
                        # o65 evacuation on ACT: the DVE is the busier
                        # engine once 3-in-8 steps take the custom-op path
                        nc.scalar.copy(
                            o65[:, w * qb : w * (qb + 1)], av[0:VW, :]
                        )
                        del avs[qb]
                        if b == 1 and qb == n_qb - 1:
                            den_jobs.append(make_den_pe(b, qb, o65, vp))
                            return
                        # denominator bounce via casting DMA (bf16 -> f32;
                        # only gpsimd may cast); latency hides under
                        # subsequent attention steps
                        nc.gpsimd.dma_start(
                            den_scr[b : b + 1, w * qb : w * (qb + 1)],
                            o65[HEAD_DIM : HEAD_DIM + 1, w * qb : w * (qb + 1)],
                        )
                        denT = recp.tile([128, qb_t], F32, tag="denT")
                        bsrc = bass.AP(
                            tensor=den_scr,
                            offset=b * n + w * qb,
                            ap=[[1, 128], [128, qb_t]],
                        )
                        nc.gpsimd.dma_start(denT, bsrc)
                        rec = recp.tile([128, qb_t], F32, tag="rec")
                        nc.vector.reciprocal(rec, denT)
                        for rj in range(qb_t):
                            proj_jobs.append((b, qb_t * qb + rj, o65, rec, rj))

                for s in range(nsteps + LAG):
                    cur_s[0] = s
                    if s < nsteps:
                        issue_qk(s)
                    if s >= LAG:
                        issue_av(s - LAG)
                    # two projection MMs share one 2-bank PSUM slot every 8
                    # steps; their ACT/DVE drains land several steps later
                    # (and only once enqueued >= 6 steps) so the in-order
                    # queues never wait on an in-flight MM or a reciprocal
                    # still deep in the DVE queue.  The last query block's
                    # jobs spill into the next batch's loop so the batch
                    # edge doesn't stall the PE queue
                    if proj_jobs and s % 8 == 1:
                        pjd = pst.tile([128, 2 * OUT_F], F32, tag="st")
                        for half in range(2):
                            if proj_jobs:
                                pb, pqt, po65, prec, prj = proj_jobs.pop(0)
                                make_pj(
                                    pb, pqt, po65, prec, prj,
                                    pair=(pjd, half),
                                )()
                    if drain_jobs and s % 8 in (3, 5):
                        drain_jobs.pop(0)[1]()

            while den_jobs:
                den_jobs.pop(0)()
            while proj_jobs:
                pb, pqt, po65, prec, prj = proj_jobs.pop(0)
                make_pj(pb, pqt, po65, prec, prj)()
                while drain_jobs:
                    drain_jobs.pop(0)[1]()

    nc.compile()
    return nc


def make_in_maps(q, k, v, rel_bias_table, W_out, n):
    """Shard + pre-layout full inputs per core (core c <-> head c)."""
    Kt = n // 128
    in_maps = []
    for c in range(N_CORES):
        sl = slice(HEAD_DIM * c, HEAD_DIM * (c + 1))
        qT = np.zeros((2, KD, n), dtype=ml_dtypes.bfloat16)
        qT[:, :HEAD_DIM] = np.transpose(
            q[:, :, sl] * np.float32(QS), (0, 2, 1)
        ).astype(ml_dtypes.bfloat16)
        qT[:, HEAD_DIM] = 1.0
        kT = np.zeros((2, KD, n), dtype=ml_dtypes.bfloat16)
        kT[:, :HEAD_DIM] = np.transpose(k[:, :, sl], (0, 2, 1)).astype(
            ml_dtypes.bfloat16
        )
        kT[:, HEAD_DIM] = np.float32(COFF)  # exact in bf16
        vr = v[:, :, sl].reshape(2, Kt, 128, HEAD_DIM)
        vp = np.zeros((2, 128, Kt, 128), dtype=ml_dtypes.bfloat16)
        vp[:, :, :, :HEAD_DIM] = np.transpose(vr, (0, 2, 1, 3)).astype(
            ml_dtypes.bfloat16
        )
        vp[:, :, :, HEAD_DIM] = 1.0
        expP, biasT = _toeplitz_tables(
            rel_bias_table[:, c].astype(np.float64), n
        )
        in_maps.append(
            {
                "qT_h": np.ascontiguousarray(qT),
                "kT_h": np.ascontiguousarray(kT),
                "vp_h": np.ascontiguousarray(vp.reshape(2, 128, Kt * 128)),
                "expP_h": expP,
                "biasT_h": biasT,
                "W_h": np.ascontiguousarray(W_out[sl, :]).astype(
                    ml_dtypes.bfloat16
                ),
            }
        )
    return in_maps


_NC_CACHE = {}


def _get_nc(n, w):
    key = (n, w)
    if key not in _NC_CACHE:
        _NC_CACHE[key] = build_nc(n=n, w=w)
    return _NC_CACHE[key]


def kernel(q, k, v, rel_bias_table, W_out, b_out):
    from concourse.bass_utils import run_bass_kernel_spmd

    q = np.asarray(q, dtype=np.float32)
    k = np.asarray(k, dtype=np.float32)
    v = np.asarray(v, dtype=np.float32)
    rel_bias_table = np.asarray(rel_bias_table, dtype=np.float32)
    W_out = np.asarray(W_out, dtype=np.float32)
    b_out = np.asarray(b_out, dtype=np.float32)

    n = q.shape[1]
    w = min(1024, n)
    nc = _get_nc(n, w)
    in_maps = make_in_maps(q, k, v, rel_bias_table, W_out, n)
    res = run_bass_kernel_spmd(nc, in_maps, core_ids=list(range(N_CORES)))
    acc = np.zeros((2, n, OUT_F), dtype=np.float64)
    for r in res.results:
        acc += r["out_partial"].astype(np.float64)
    return (acc + b_out.astype(np.float64)).astype(np.float32)


# revision 25
# speedup vs baseline: 1.0106x; 1.0106x over previous
"""Trainium2 Bass kernel for nn_AttentionBase (8-head attention w/ T5-style
relative-position bias + output projection), sharded head-parallel over 8
NeuronCores.

v4: the n^2 softmax-exp elementwise work is split between the ACT engine
(exact spline exp, A-steps) and the DVE (custom fused fast-exp2 op,
D-steps, every 3rd step) so both engines stream PSUM score tiles in
parallel instead of ACT being the lone ~288us bottleneck of v1.

Host side (per core c, head h=c): q is pre-scaled by QS = 128*log2(e)*SCALE
so the QK PSUM output is in "bf16 bits" units; both q,k get a 65th
contraction row (qT=1, kT=16256) so st arrives with the +16256 bf16
exponent-bias offset pre-added (16256 = 127*128 is exact in bf16).
v is laid out as [2, 128, Kt*128] bf16 with a ones-column at 64
(softmax-denominator trick). Two Toeplitz tables:
  expP  [128, 2n-128] f16 : kappa * exp(SCALE*bias[key-query])   (A-steps)
  biasT [128, 2n-128] f16 : QS*bias[key-query] + delta           (D-steps)

Device program per core (single head, both batches), per step s=(qb, kt):
  S^T[128, w] = kT_tile^T.T @ qT             (PE, bf16 in, fp32 PSUM,
                                              3-deep st ring)
  A-step (2 of 3): araw = exp(S*SCALE/QS - 16256*SCALE/QS)  (ACT spline)
                   at = araw * expP_slice    (DVE tensor_tensor, 2x 16-bit)
  D-step (1 of 3): at.bits = EXP2BITS(S^T + biasT_slice)
                   (custom DVE op: u+=bias; round-to-128-multiple via +-2^30;
                    quadratic mantissa fit; int16 convert == exact bf16 bits
                    of kappa*exp(SCALE*(s+bias)); one 1x DVE pass, no ACT)
  outT[65, w] += V'[kt]^T.T @ at             (PE, lagged LAG steps)
Projections (o65 chunk @ W_h -> PSUM, two per shared 2-bank st-ring slot
every 8 steps) drain via ACT scalar.mul(1/den) several steps after their
matmul — and only once their 1/den reciprocal has had time to clear the DVE
queue — so the in-order ACT queue never stalls on an in-flight producer;
the kappa factor cancels in the softmax normalization.
Host: out = sum_c partial_c + b_out (float64 accumulate).
"""

import math
import sys

sys.path.insert(0, "/opt/trn_rl_repo")

import numpy as np
import ml_dtypes

import concourse.bass as bass
import concourse.bacc as bacc_mod
import concourse.mybir as mybir
import concourse.tile as tile

NUM_HEADS = 8
HEAD_DIM = 64
MID = 512
OUT_F = 512
NUM_BUCKETS = 32
MAX_DISTANCE = 128
SCALE = HEAD_DIM ** -0.5
N_CORES = 8

F32 = mybir.dt.float32
BF16 = mybir.dt.bfloat16
F16 = mybir.dt.float16
I16 = mybir.dt.int16
AF = mybir.ActivationFunctionType

# fast-exp2 constants (round-to-nearest-even int16 convert, HW-verified)
QS = 128 * math.log2(math.e) * SCALE        # 23.083120654223414 bits/unit
COFF = 16256.0                              # 127*128, exact in bf16
EXP2_DELTA = 24.04
EXP2_B = 0.0027580
EXP2_D = 53.08
EXP2_KAPPA = 1.611958
M30 = float(2.0 ** 30)
DK = 3                                       # every DK-th step is a D-step
KD = 65                                      # contraction: 64 dims + COFF row


def _register_exp2_op():
    """Idempotently register the custom DVE op (documented extension path:
    dve_ops.OPS + sub-opcode row; the per-NEFF uop table is generated at
    compile time, no firmware change)."""
    import concourse.dve_ops as dve_ops
    from concourse.dve_spec import Spec, Src0, Src1, C0, C1, C2

    for op in dve_ops.OPS:
        if op.name == "EXP2BITS_ANT":
            return op

    def ref(in0, in1, c0, c1, c2):
        c0v = np.float32(c0 if np.isscalar(c0) else np.asarray(c0).ravel()[0])
        c1v = np.float32(c1 if np.isscalar(c1) else np.asarray(c1).ravel()[0])
        u = (in0.astype(np.float32) + in1.astype(np.float32)).astype(np.float32)
        t = (u + c0v).astype(np.float32)
        i = (t - c0v).astype(np.float32)
        f = (u - i).astype(np.float32)
        p = ((np.float32(c2) * f) * f + c1v).astype(np.float32)
        return (u + p).astype(np.float32)

    u = Src0 + Src1
    t = u + C0
    i = t - C0
    f = u - i
    body = u + ((C2 * f) * f + C1)
    op = dve_ops.DveOp(
        "EXP2BITS_ANT",
        dve_ops.Spec(body=body, reference=ref),
        subdim=False,
        uops_sha={"v3": "3f0b102a66e52a77"},
    )
    dve_ops.OPS.append(op)
    dve_ops.CUSTOM_DVE_SPECS[op.name] = op.spec
    dve_ops._SUB_OPCODE_FOR_NAME[op.name] = (
        dve_ops._CUSTOM_DVE_ROW_BASE + len(dve_ops.OPS) - 1
    )
    assert dve_ops._SUB_OPCODE_FOR_NAME[op.name] < 0x20
    return op


EXP2_OP = _register_exp2_op()


def _bucket_np(rel):
    """Exact numpy port of reference._relative_position_bucket with
    num_buckets=64, max_distance=128 (as the module calls it)."""
    num_buckets = (2 * NUM_BUCKETS) // 2  # 32
    ret = (rel >= 0).astype(np.int32) * num_buckets
    n = np.abs(rel)
    max_exact = max(1, num_buckets // 2)  # 16
    denom = (
        math.log(MAX_DISTANCE / max_exact) if MAX_DISTANCE > max_exact else 1.0
    )
    n_float = np.maximum(n.astype(np.float32), 1.0)
    val_if_large = (
        max_exact + np.log(n_float / max_exact) / denom * (num_buckets - max_exact)
    ).astype(np.int32)
    val_if_large = np.minimum(val_if_large, num_buckets - 1)
    return ret + np.where(n < max_exact, n, val_if_large)


def _toeplitz_tables(table_col, n):
    """expP (f16) and biasT (f16) [128, 2n-128] Toeplitz tables. Slice
    [:, c0:c0+w] with c0 = (n-128) - 128*kt + w*qb gives the [key, query]
    tile."""
    x = np.arange(2 * n, dtype=np.int64)
    d = (n - 1) - x
    bias = table_col[_bucket_np(d)].astype(np.float64)
    expP_seq = (EXP2_KAPPA * np.exp(SCALE * bias)).astype(np.float16)
    biasT_seq = (QS * bias + EXP2_DELTA).astype(np.float16)
    Wp = 2 * n - 128
    idx = (127 - np.arange(128, dtype=np.int64))[:, None] + np.arange(
        Wp, dtype=np.int64
    )[None, :]
    return expP_seq[idx], biasT_seq[idx]


def build_nc(n=4096, w=1024):
    assert n % 128 == 0 and n % w == 0
    Kt = n // 128           # key tiles
    n_qb = n // w           # query blocks per batch
    nsteps = n_qb * Kt      # attention steps per batch
    qb_t = w // 128         # query tiles per query block
    Wp = 2 * n - 128
    LAG = 4                 # AV trails QK by this many steps on the PE queue
    VW = HEAD_DIM + 1       # V' valid width (ones column at 64)
    VP = 128                # V' padded width: 128 weights enables FWL

    nc = bacc_mod.Bacc()
    qT_h = nc.declare_dram_parameter("qT_h", [2, KD, n], BF16, isOutput=False)
    kT_h = nc.declare_dram_parameter("kT_h", [2, KD, n], BF16, isOutput=False)
    vp_h = nc.declare_dram_parameter("vp_h", [2, 128, Kt * VP], BF16, isOutput=False)
    expP_h = nc.declare_dram_parameter("expP_h", [128, Wp], F16, isOutput=False)
    biasT_h = nc.declare_dram_parameter("biasT_h", [128, Wp], F16, isOutput=False)
    W_h = nc.declare_dram_parameter("W_h", [HEAD_DIM, OUT_F], BF16, isOutput=False)
    out_p = nc.declare_dram_parameter(
        "out_partial", [2, n, OUT_F], BF16, isOutput=True
    )
    den_scr = nc.dram_tensor("den_scr", (2, n), F32)

    act_scale = float(SCALE / QS)
    act_bias = float(-COFF * SCALE / QS)

    def is_d_step(b, s):
        return s % DK == DK - 1

    with tile.TileContext(nc) as tc:
        with (
            tc.tile_pool(name="const", bufs=1) as constp,
            tc.tile_pool(name="qkT", bufs=2) as qkTp,
            tc.tile_pool(name="vpp", bufs=2) as vpp,
            tc.tile_pool(name="o65p", bufs=2) as o65p,
            tc.tile_pool(name="stage", bufs=6) as stage,
            tc.tile_pool(name="atp", bufs=LAG + 4) as atp,
            tc.tile_pool(name="osbp", bufs=6) as osbp,
            tc.tile_pool(name="recp", bufs=3) as recp,
            tc.tile_pool(name="pst", bufs=3, space="PSUM") as pst,
            tc.tile_pool(name="pav", bufs=1, space="PSUM") as pav,
        ):
            # PE warm-up burst: a few dependency-free matmuls on a memset
            # tile keep the PE fed while the first input chunks stream in
            warm = constp.tile([HEAD_DIM, 512], BF16, tag="warm")
            nc.gpsimd.memset(warm, 0.0)
            # per-partition constant for the A-step ACT bias (-COFF*SCALE/QS)
            actb = constp.tile([128, 1], F32, tag="actb")
            nc.gpsimd.memset(actb, act_bias)
            wpsum = pst.tile([128, 512], F32, tag="st", name="wpsum")
            for _ in range(4):
                nc.tensor.matmul(
                    wpsum, warm[:, 0:128], warm[:, :], start=True, stop=True
                )

            # Toeplitz tables stream on the sync queue in first-use order
            # (the first steps read cols ~[n-128-256, n-128+w)); chunked so
            # the first A/D steps wait only on their own chunk.
            expP = constp.tile([128, Wp], F16, tag="expP")
            biasT = constp.tile([128, Wp], F16, tag="biasT")
            # first-use priority: qb0 reads cols [n-128+w) downward from
            # c0 = n-128-128*kt, so the top-of-range slices gate step 0
            hi0 = n - 128 + w
            mid = (hi0 + Wp) // 2
            for c0, c1 in (
                (n - 512, hi0),
                (n // 2, n - 512),
                (0, n // 2),
                (hi0, mid),
                (mid, Wp),
            ):
                cs = slice(c0, c1)
                nc.sync.dma_start(expP[:, cs], expP_h[:, cs])
                nc.sync.dma_start(biasT[:, cs], biasT_h[:, cs])
            Wt = constp.tile([HEAD_DIM, OUT_F], BF16, tag="W")
            nc.sync.dma_start(Wt, W_h[:, :])

            proj_jobs = []   # pending projections (args tuples)
            drain_jobs = []  # (enqueue_step, drain closure)
            den_jobs = []    # deferred final-block denominator closure
            cur_s = [10 ** 9]  # current step, for drain readiness

            def make_den_pe(b, qb, o65, vp):
                # transpose the [1, w] denominator row to [128, qb_t] with
                # qb_t rank-1 PE matmuls — used only for the very last query
                # block, where the DRAM bounce's serial latency would sit
                # entirely in the kernel tail
                def issue():
                    denP = pst.tile([128, qb_t], F32, tag="st", name="denP")
                    for j in range(qb_t):
                        nc.tensor.matmul(
                            denP[:, j : j + 1],
                            o65[
                                HEAD_DIM : HEAD_DIM + 1,
                                w * qb + 128 * j : w * qb + 128 * (j + 1),
                            ],
                            vp[
                                HEAD_DIM : HEAD_DIM + 1,
                                HEAD_DIM : HEAD_DIM + 1,
                            ],
                            start=True,
                            stop=True,
                        )
                    denT = recp.tile([128, qb_t], F32, tag="denT")
                    nc.vector.tensor_copy(denT, denP)
                    rec = recp.tile([128, qb_t], F32, tag="rec")
                    nc.vector.reciprocal(rec, denT)
                    # tail: issue all MMs first, then drains (alternating
                    # engines) so no drain waits on an in-flight MM
                    local = []
                    for rj in range(qb_t):
                        local.append(
                            make_pj(
                                b, qb_t * qb + rj, o65, rec, rj,
                                on_dve=rj % 2 == 1,
                            )
                        )
                    for j, mm in enumerate(local):
                        mm()
                        if j >= 1:
                            drain_jobs.pop(0)[1]()
                    while drain_jobs:
                        drain_jobs.pop(0)[1]()

                return issue

            def make_pj(b, qt, o65, rec, rj, on_dve=False, pair=None):
                """Projection for query tile qt. `pair`: (pj_tile, half) to
                share one 2-bank PSUM slot between two projections."""

                def issue_mm():
                    if pair is None:
                        pj = pst.tile([128, OUT_F], F32, tag="st")
                        lo = 0
                    else:
                        pjt, half = pair
                        pj = pjt
                        lo = OUT_F * half
                    nc.tensor.matmul(
                        pj[:, lo : lo + OUT_F],
                        o65[0:HEAD_DIM, 128 * qt : 128 * (qt + 1)],
                        Wt,
                        start=True,
                        stop=True,
                    )

                    def issue_drain():
                        osb = osbp.tile([128, OUT_F], BF16, tag="osb")
                        src = pj[:, lo : lo + OUT_F]
                        if on_dve:
                            nc.vector.tensor_scalar_mul(
                                osb, src, rec[:, rj : rj + 1]
                            )
                        else:
                            # steady state: ACT has slack, DVE is busier
                            nc.scalar.mul(osb, src, rec[:, rj : rj + 1])
                        nc.sync.dma_start(
                            out_p[b, 128 * qt : 128 * (qt + 1), :], osb
                        )

                    drain_jobs.append((cur_s[0], issue_drain))

                return issue_mm

            for b in range(2):
                # loads ordered by first use: the first key tiles and the
                # first query block gate step 0; the rest streams behind
                qT = qkTp.tile([KD, n], BF16, tag="qT")
                kT = qkTp.tile([KD, n], BF16, tag="kT")
                nc.gpsimd.dma_start(kT[:, 0:512], kT_h[b][:, 0:512])
                nc.gpsimd.dma_start(qT[:, 0:w], qT_h[b][:, 0:w])
                nc.gpsimd.dma_start(kT[:, 512:2048], kT_h[b][:, 512:2048])
                nc.gpsimd.dma_start(kT[:, 2048:n], kT_h[b][:, 2048:n])
                vp = vpp.tile([128, Kt * VP], BF16, tag="vp")
                nc.gpsimd.dma_start(
                    vp[:, 0 : Kt * VP // 4], vp_h[b][:, 0 : Kt * VP // 4]
                )
                if n > w:
                    nc.gpsimd.dma_start(qT[:, w:n], qT_h[b][:, w:n])
                for ch in range(1, 4):
                    cs = slice(ch * Kt * VP // 4, (ch + 1) * Kt * VP // 4)
                    nc.gpsimd.dma_start(vp[:, cs], vp_h[b][:, cs])

                o65 = o65p.tile([VW, n], BF16, tag="o65")
                ats = {}
                avs = {}

                def issue_qk(s, b=b, qT=qT, kT=kT, ats=ats):
                    qb, kt = divmod(s, Kt)
                    st = pst.tile([128, w], F32, tag="st")
                    for h in range(w // 512):
                        nc.tensor.matmul(
                            st[:, 512 * h : 512 * (h + 1)],
                            kT[:, 128 * kt : 128 * (kt + 1)],
                            qT[:, w * qb + 512 * h : w * qb + 512 * (h + 1)],
                            start=True,
                            stop=True,
                        )
                    at = atp.tile([128, w], BF16, tag="at")
                    c0 = (n - 128) - 128 * kt + w * qb
                    if is_d_step(b, s):
                        nc.vector._custom_dve(
                            EXP2_OP,
                            out=at[:, :].bitcast(I16),
                            in0=st[:, :],
                            in1=biasT[:, c0 : c0 + w],
                            s0=M30,
                            s1=EXP2_D,
                            imm2=EXP2_B,
                        )
                    else:
                        araw = stage.tile([128, w], BF16, tag="araw")
                        nc.scalar.activation(
                            araw, st, AF.Exp, bias=actb[:, 0:1], scale=act_scale
                        )
                        nc.vector.tensor_mul(at, araw, expP[:, c0 : c0 + w])
                    ats[s] = at

                def issue_av(s, b=b, vp=vp, o65=o65, ats=ats, avs=avs):
                    qb, kt = divmod(s, Kt)
                    if kt == 0:
                        avs[qb] = pav.tile([128, w], F32, tag="av", name="av")
                    av = avs[qb]
                    at = ats.pop(s)
                    for h in range(w // 512):
                        nc.tensor.matmul(
                            av[:, 512 * h : 512 * (h + 1)],
                            vp[:, VP * kt : VP * (kt + 1)],
                            at[:, 512 * h : 512 * (h + 1)],
                            start=(kt == 0),
                            stop=(kt == Kt - 1),
                        )
                    if kt == Kt - 1:
                        nc.vector.tensor_copy(
                            o65[:, w * qb : w * (qb + 1)], av[0:VW, :]
                        )
                        del avs[qb]
                        if b == 1 and qb == n_qb - 1:
                            den_jobs.append(make_den_pe(b, qb, o65, vp))
                            return
                        # denominator bounce via casting DMA (bf16 -> f32;
                        # only gpsimd may cast); latency hides under
                        # subsequent attention steps
                        nc.gpsimd.dma_start(
                            den_scr[b : b + 1, w * qb : w * (qb + 1)],
                            o65[HEAD_DIM : HEAD_DIM + 1, w * qb : w * (qb + 1)],
                        )
                        denT = recp.tile([128, qb_t], F32, tag="denT")
                        bsrc = bass.AP(
                            tensor=den_scr,
                            offset=b * n + w * qb,
                            ap=[[1, 128], [128, qb_t]],
                        )
                        nc.gpsimd.dma_start(denT, bsrc)
                        rec = recp.tile([128, qb_t], F32, tag="rec")
                        nc.vector.reciprocal(rec, denT)
                        for rj in range(qb_t):
                            proj_jobs.append((b, qb_t * qb + rj, o65, rec, rj))

                for s in range(nsteps + LAG):
                    cur_s[0] = s
                    if s < nsteps:
                        issue_qk(s)
                    if s >= LAG:
                        issue_av(s - LAG)
                    # two projection MMs share one 2-bank PSUM slot every 8
                    # steps; their ACT/DVE drains land several steps later
                    # (and only once enqueued >= 6 steps) so the in-order
                    # queues never wait on an in-flight MM or a reciprocal
                    # still deep in the DVE queue.  The last query block's
                    # jobs spill into the next batch's loop so the batch
                    # edge doesn't stall the PE queue
                    if proj_jobs and s % 8 == 1:
                        pjd = pst.tile([128, 2 * OUT_F], F32, tag="st")
                        for half in range(2):
                            if proj_jobs:
                                pb, pqt, po65, prec, prj = proj_jobs.pop(0)
                                make_pj(
                                    pb, pqt, po65, prec, prj,
                                    pair=(pjd, half),
                                )()
                    if drain_jobs and s % 8 in (3, 5):
                        drain_jobs.pop(0)[1]()

            while den_jobs:
                den_jobs.pop(0)()
            while proj_jobs:
                pb, pqt, po65, prec, prj = proj_jobs.pop(0)
                make_pj(pb, pqt, po65, prec, prj)()
                while drain_jobs:
                    drain_jobs.pop(0)[1]()

    nc.compile()
    return nc


def make_in_maps(q, k, v, rel_bias_table, W_out, n):
    """Shard + pre-layout full inputs per core (core c <-> head c)."""
    Kt = n // 128
    in_maps = []
    for c in range(N_CORES):
        sl = slice(HEAD_DIM * c, HEAD_DIM * (c + 1))
        qT = np.zeros((2, KD, n), dtype=ml_dtypes.bfloat16)
        qT[:, :HEAD_DIM] = np.transpose(
            q[:, :, sl] * np.float32(QS), (0, 2, 1)
        ).astype(ml_dtypes.bfloat16)
        qT[:, HEAD_DIM] = 1.0
        kT = np.zeros((2, KD, n), dtype=ml_dtypes.bfloat16)
        kT[:, :HEAD_DIM] = np.transpose(k[:, :, sl], (0, 2, 1)).astype(
            ml_dtypes.bfloat16
        )
        kT[:, HEAD_DIM] = np.float32(COFF)  # exact in bf16
        vr = v[:, :, sl].reshape(2, Kt, 128, HEAD_DIM)
        vp = np.zeros((2, 128, Kt, 128), dtype=ml_dtypes.bfloat16)
        vp[:, :, :, :HEAD_DIM] = np.transpose(vr, (0, 2, 1, 3)).astype(
            ml_dtypes.bfloat16
        )
        vp[:, :, :, HEAD_DIM] = 1.0
        expP, biasT = _toeplitz_tables(
            rel_bias_table[:, c].astype(np.float64), n
        )
        in_maps.append(
            {
                "qT_h": np.ascontiguousarray(qT),
                "kT_h": np.ascontiguousarray(kT),
                "vp_h": np.ascontiguousarray(vp.reshape(2, 128, Kt * 128)),
                "expP_h": expP,
                "biasT_h": biasT,
                "W_h": np.ascontiguousarray(W_out[sl, :]).astype(
                    ml_dtypes.bfloat16
                ),
            }
        )
    return in_maps


_NC_CACHE = {}


def _get_nc(n, w):
    key = (n, w)
    if key not in _NC_CACHE:
        _NC_CACHE[key] = build_nc(n=n, w=w)
    return _NC_CACHE[key]


def kernel(q, k, v, rel_bias_table, W_out, b_out):
    from concourse.bass_utils import run_bass_kernel_spmd

    q = np.asarray(q, dtype=np.float32)
    k = np.asarray(k, dtype=np.float32)
    v = np.asarray(v, dtype=np.float32)
    rel_bias_table = np.asarray(rel_bias_table, dtype=np.float32)
    W_out = np.asarray(W_out, dtype=np.float32)
    b_out = np.asarray(b_out, dtype=np.float32)

    n = q.shape[1]
    w = min(1024, n)
    nc = _get_nc(n, w)
    in_maps = make_in_maps(q, k, v, rel_bias_table, W_out, n)
    res = run_bass_kernel_spmd(nc, in_maps, core_ids=list(range(N_CORES)))
    acc = np.zeros((2, n, OUT_F), dtype=np.float64)
    for r in res.results:
        acc += r["out_partial"].astype(np.float64)
    return (acc + b_out.astype(np.float64)).astype(np.float32)
